# revision 7
# baseline (speedup 1.0000x reference)
"""EquiConv (DeepH-E3) Trainium2 kernel — 8-core data-parallel over edges.

Strategy (channel-major on device):
  - Host folds all per-channel weights/constants into matmul weight
    matrices, shards edges across 8 cores, pads to a multiple of 512 and
    transposes edge tensors to channel-major [C, E].
  - Device processes 512-edge tiles: per-edge scalars (x2s, x2v_i) are
    replicated across partitions with a K=1 float32r matmul (ones-trick),
    activations are pre-scaled on DVE, all tensor-product paths become
    accumulating float32r matmuls into PSUM, Gate nonlinearity uses
    Silu/Tanh (one ACT table set; sigmoid = 0.5*tanh(x/2)+0.5), and the
    e3ElementWise multiply is fused into the output elementwise ops.
  - Host transposes the [320, E] channel-major output back.

Self-contained: hardcodes shapes from the problem spec; no file reads.
"""
import os
import sys

import numpy as np

# ---------------------------------------------------------------- constants
E_FULL = 200000
N_CORES = 8
E_CORE = E_FULL // N_CORES      # 25000
NT = 512                        # edges per tile
T_TILES = 49                    # tiles per core
E_PAD = NT * T_TILES            # 25088
MUL_S = 128
MUL_V = 64

INV_S = 1.0 / np.sqrt(MUL_S)
INV_V = 1.0 / np.sqrt(MUL_V)
SQ2 = 1.0 / np.sqrt(2.0)
SQ3 = 1.0 / np.sqrt(3.0)

_REPO_CANDIDATES = (
    "/opt/trn_rl_repo",
    "/root/.axon_site/_ro/trn_rl_repo",
)


def _ensure_repo_on_path():
    try:
        import concourse.bass  # noqa: F401
        return
    except ImportError:
        pass
    for p in _REPO_CANDIDATES:
        if os.path.isdir(p) and p not in sys.path:
            sys.path.insert(0, p)
    import concourse.bass  # noqa: F401


_CACHE = {}


def _build_nc():
    """Build + compile the per-core Bass program (cached)."""
    if "nc" in _CACHE:
        return _CACHE["nc"]
    _ensure_repo_on_path()
    import concourse.mybir as mybir
    import concourse.tile as tile
    from concourse import bacc

    F32 = mybir.dt.float32
    F32R = mybir.dt.float32r
    MULT = mybir.AluOpType.mult
    ADD = mybir.AluOpType.add
    AF = mybir.ActivationFunctionType

    nc = bacc.Bacc(trn_type="TRN2", target_bir_lowering=False, debug=False,
                   num_devices=N_CORES)

    # DRAM inputs (per-core shard, channel-major) --------------------------
    d_x1s = nc.dram_tensor("x1s_t", [128, E_PAD], F32R, kind="ExternalInput")
    d_x1v = nc.dram_tensor("x1v_t", [192, E_PAD], F32R, kind="ExternalInput")
    d_x2 = nc.dram_tensor("x2_tiled", [T_TILES, 4 * NT], F32R,
                          kind="ExternalInput")
    d_fw = nc.dram_tensor("fw_t", [128, E_PAD], F32R, kind="ExternalInput")
    # folded weights ([K, M] layouts, ready as lhsT)
    d_wa0 = nc.dram_tensor("wa0", [128, 128], F32R, kind="ExternalInput")
    d_wa1 = nc.dram_tensor("wa1", [128, 64], F32R, kind="ExternalInput")
    d_wp2 = nc.dram_tensor("wp2", [128, 64], F32R, kind="ExternalInput")
    d_wb4b = nc.dram_tensor("wb4b", [64, 128], F32R, kind="ExternalInput")
    d_wb5b = nc.dram_tensor("wb5b", [64, 64], F32R, kind="ExternalInput")
    d_wc = nc.dram_tensor("wc", [64, 64], F32R, kind="ExternalInput")
    d_fc0 = nc.dram_tensor("fc0", [128, 64], F32R, kind="ExternalInput")
    d_fc1 = nc.dram_tensor("fc1", [64, 64], F32R, kind="ExternalInput")
    d_fc2a = nc.dram_tensor("fc2a", [64, 128], F32R, kind="ExternalInput")
    d_fc2b = nc.dram_tensor("fc2b", [64, 64], F32R, kind="ExternalInput")
    d_b0 = nc.dram_tensor("b0c", [64, 1], F32, kind="ExternalInput")
    d_b1 = nc.dram_tensor("b1c", [64, 1], F32, kind="ExternalInput")
    d_b2a = nc.dram_tensor("b2a", [128, 1], F32, kind="ExternalInput")
    d_b2b = nc.dram_tensor("b2v", [64, 1], F32, kind="ExternalInput")
    d_ones = nc.dram_tensor("ones_d", [1, 128], F32R, kind="ExternalInput")

    d_out = nc.dram_tensor("out_t", [320, E_PAD], F32, kind="ExternalOutput")

    with tile.TileContext(nc) as tc:
        with tc.tile_pool(name="const", bufs=1) as cp, \
             tc.tile_pool(name="io", bufs=3) as io, \
             tc.tile_pool(name="work", bufs=2) as wk, \
             tc.tile_pool(name="ps", bufs=1, space="PSUM") as ps:

            # constants into SBUF once
            def const(d, shape, dtype=F32R, name=None):
                t = cp.tile(shape, dtype, name=name or d.name + "_sb")
                nc.sync.dma_start(t, d.ap())
                return t

            w_wa0 = const(d_wa0, [128, 128])
            w_wa1 = const(d_wa1, [128, 64])
            w_wp2 = const(d_wp2, [128, 64])
            w_wb4b = const(d_wb4b, [64, 128])
            w_wb5b = const(d_wb5b, [64, 64])
            w_wc = const(d_wc, [64, 64])
            w_fc0 = const(d_fc0, [128, 64])
            w_fc1 = const(d_fc1, [64, 64])
            w_fc2a = const(d_fc2a, [64, 128])
            w_fc2b = const(d_fc2b, [64, 64])
            c_b0 = const(d_b0, [64, 1], F32)
            c_b1 = const(d_b1, [64, 1], F32)
            c_b2a = const(d_b2a, [128, 1], F32)
            c_b2b = const(d_b2b, [64, 1], F32)
            c_ones = const(d_ones, [1, 128])

            for t in range(T_TILES):
                sl = slice(t * NT, (t + 1) * NT)

                # ---- loads -------------------------------------------
                x1s = io.tile([128, NT], F32R)
                nc.sync.dma_start(x1s, d_x1s.ap()[:, sl])
                x1vc0 = io.tile([64, NT], F32R)
                nc.sync.dma_start(x1vc0, d_x1v.ap()[0:64, sl])
                x1vc1 = io.tile([64, NT], F32R)
                nc.sync.dma_start(x1vc1, d_x1v.ap()[64:128, sl])
                x1vc2 = io.tile([64, NT], F32R)
                nc.sync.dma_start(x1vc2, d_x1v.ap()[128:192, sl])
                x2sb = io.tile([1, 4 * NT], F32R)
                nc.sync.dma_start(x2sb, d_x2.ap()[t:t + 1, :])
                fwt = io.tile([128, NT], F32R)
                nc.sync.dma_start(fwt, d_fw.ap()[:, sl])

                # ---- per-edge scalar replication + prescales ---------
                # x2s
                rep_s = ps.tile([128, NT], F32, tag="rep")
                nc.tensor.matmul(rep_s, c_ones, x2sb[:, 0:NT],
                                 start=True, stop=True)
                x1s_s = wk.tile([128, NT], F32R)
                nc.vector.tensor_tensor(x1s_s, x1s, rep_s, MULT)
                xv_s0 = wk.tile([64, NT], F32R)
                nc.vector.tensor_tensor(xv_s0, x1vc0, rep_s[0:64, :], MULT)
                xv_s1 = wk.tile([64, NT], F32R)
                nc.vector.tensor_tensor(xv_s1, x1vc1, rep_s[0:64, :], MULT)
                xv_s2 = wk.tile([64, NT], F32R)
                nc.vector.tensor_tensor(xv_s2, x1vc2, rep_s[0:64, :], MULT)
                # x2v_0
                rep_v0 = ps.tile([128, NT], F32, tag="rep")
                nc.tensor.matmul(rep_v0, c_ones, x2sb[:, NT:2 * NT],
                                 start=True, stop=True)
                x1s_v0 = wk.tile([128, NT], F32R)
                nc.vector.tensor_tensor(x1s_v0, x1s, rep_v0, MULT)
                xv_p0 = wk.tile([64, NT], F32R)
                nc.vector.tensor_tensor(xv_p0, x1vc0, rep_v0[0:64, :], MULT)
                # x2v_1
                rep_v1 = ps.tile([128, NT], F32, tag="rep")
                nc.tensor.matmul(rep_v1, c_ones, x2sb[:, 2 * NT:3 * NT],
                                 start=True, stop=True)
                x1s_v1 = wk.tile([128, NT], F32R)
                nc.vector.tensor_tensor(x1s_v1, x1s, rep_v1, MULT)
                xv_p1 = wk.tile([64, NT], F32R)
                nc.vector.tensor_tensor(xv_p1, x1vc1, rep_v1[0:64, :], MULT)
                # x2v_2
                rep_v2 = ps.tile([128, NT], F32, tag="rep")
                nc.tensor.matmul(rep_v2, c_ones, x2sb[:, 3 * NT:4 * NT],
                                 start=True, stop=True)
                x1s_v2 = wk.tile([128, NT], F32R)
                nc.vector.tensor_tensor(x1s_v2, x1s, rep_v2, MULT)
                xv_p2 = wk.tile([64, NT], F32R)
                nc.vector.tensor_tensor(xv_p2, x1vc2, rep_v2[0:64, :], MULT)

                # ---- tensor-product matmuls --------------------------
                scal = ps.tile([128, NT], F32, tag="scal")
                nc.tensor.matmul(scal, w_wa0, x1s_s, start=True, stop=False)
                nc.tensor.matmul(scal, w_wb4b, xv_p0, start=False, stop=False)
                nc.tensor.matmul(scal, w_wb4b, xv_p1, start=False, stop=False)
                nc.tensor.matmul(scal, w_wb4b, xv_p2, start=False, stop=True)

                gate = ps.tile([64, NT], F32, tag="gate")
                nc.tensor.matmul(gate, w_wa1, x1s_s, start=True, stop=False)
                nc.tensor.matmul(gate, w_wb5b, xv_p0, start=False, stop=False)
                nc.tensor.matmul(gate, w_wb5b, xv_p1, start=False, stop=False)
                nc.tensor.matmul(gate, w_wb5b, xv_p2, start=False, stop=True)

                vec0 = ps.tile([64, NT], F32, tag="vec0")
                nc.tensor.matmul(vec0, w_wp2, x1s_v0, start=True, stop=False)
                nc.tensor.matmul(vec0, w_wc, xv_s0, start=False, stop=True)
                vec1 = ps.tile([64, NT], F32, tag="vec1")
                nc.tensor.matmul(vec1, w_wp2, x1s_v1, start=True, stop=False)
                nc.tensor.matmul(vec1, w_wc, xv_s1, start=False, stop=True)
                vec2 = ps.tile([64, NT], F32, tag="vec2")
                nc.tensor.matmul(vec2, w_wp2, x1s_v2, start=True, stop=False)
                nc.tensor.matmul(vec2, w_wc, xv_s2, start=False, stop=True)

                # ---- radial MLP --------------------------------------
                h1 = ps.tile([64, NT], F32, tag="mlp")
                nc.tensor.matmul(h1, w_fc0, fwt, start=True, stop=True)
                h1s = wk.tile([64, NT], F32R)
                nc.scalar.activation(h1s, h1, AF.Silu, bias=c_b0)
                h2 = ps.tile([64, NT], F32, tag="mlp")
                nc.tensor.matmul(h2, w_fc1, h1s, start=True, stop=True)
                h2s = wk.tile([64, NT], F32R)
                nc.scalar.activation(h2s, h2, AF.Silu, bias=c_b1)
                wwa = ps.tile([128, NT], F32, tag="ww")
                nc.tensor.matmul(wwa, w_fc2a, h2s, start=True, stop=True)
                wwa_s = wk.tile([128, NT], F32)
                nc.scalar.activation(wwa_s, wwa, AF.Identity, bias=c_b2a)
                wwb = ps.tile([64, NT], F32, tag="ww")
                nc.tensor.matmul(wwb, w_fc2b, h2s, start=True, stop=True)
                wwb_s = wk.tile([64, NT], F32)
                nc.scalar.activation(wwb_s, wwb, AF.Identity, bias=c_b2b)

                # ---- gate + e3ElementWise ----------------------------
                sc_silu = wk.tile([128, NT], F32)
                nc.scalar.activation(sc_silu, scal, AF.Silu)
                tgate = wk.tile([64, NT], F32)
                nc.scalar.activation(tgate, gate, AF.Tanh, scale=0.5)
                sig = wk.tile([64, NT], F32)
                nc.vector.tensor_scalar(sig, tgate, 0.5, 0.5, MULT, ADD)
                sgw = wk.tile([64, NT], F32)
                nc.gpsimd.tensor_tensor(sgw, sig, wwb_s, MULT)

                out_s = wk.tile([128, NT], F32)
                nc.gpsimd.tensor_tensor(out_s, sc_silu, wwa_s, MULT)
                out0 = wk.tile([64, NT], F32)
                nc.vector.tensor_tensor(out0, vec0, sgw, MULT)
                out1 = wk.tile([64, NT], F32)
                nc.vector.tensor_tensor(out1, vec1, sgw, MULT)
                out2 = wk.tile([64, NT], F32)
                nc.vector.tensor_tensor(out2, vec2, sgw, MULT)

                # ---- stores ------------------------------------------
                nc.sync.dma_start(d_out.ap()[0:128, sl], out_s)
                nc.sync.dma_start(d_out.ap()[128:192, sl], out0)
                nc.sync.dma_start(d_out.ap()[192:256, sl], out1)
                nc.sync.dma_start(d_out.ap()[256:320, sl], out2)

    nc.compile()
    _CACHE["nc"] = nc
    return nc


def _fold_weights(inp):
    """Fold per-channel weights + constants into matmul matrices."""
    f = lambda k: np.asarray(inp[k], dtype=np.float32)
    w0f = f("w1_p0") * f("w2_p0")[None, :] * (INV_S * SQ2)
    w1f = f("w1_p1") * f("w2_p1")[None, :] * (INV_S * SQ2)
    w2f = f("w1_p2") * f("w2_p2")[None, :] * (INV_S * SQ2)
    w3f = f("w1_p3") * f("w2_p3")[None, :] * (INV_V * SQ2)
    w4f = f("w1_p4") * f("w2_p4")[None, :] * (INV_V * SQ3 * SQ2)
    w5f = f("w1_p5") * f("w2_p5")[None, :] * (INV_V * SQ3 * SQ2)
    fc2 = f("fc_w2")
    b2 = f("fc_b2")
    c = np.ascontiguousarray
    return {
        "wa0": c(w0f),
        "wa1": c(w1f),
        "wp2": c(w2f),
        "wb4b": c(w4f),
        "wb5b": c(w5f),
        "wc": c(w3f),
        "fc0": c(f("fc_w0")),
        "fc1": c(f("fc_w1")),
        "fc2a": c(fc2[:, :128]),
        "fc2b": c(fc2[:, 128:]),
        "b0c": c(f("fc_b0")[:, None]),
        "b1c": c(f("fc_b1")[:, None]),
        "b2a": c(b2[:128, None]),
        "b2v": c(b2[128:, None]),
        "ones_d": np.ones((1, 128), np.float32),
    }


def _shard_inputs(inp):
    """Per-core channel-major shards (padded to E_PAD edges)."""
    fea_in1 = np.asarray(inp["fea_in1"], dtype=np.float32)
    fea_in2 = np.asarray(inp["fea_in2"], dtype=np.float32)
    fea_w = np.asarray(inp["fea_weight"], dtype=np.float32)
    shards = []
    for c in range(N_CORES):
        s = slice(c * E_CORE, (c + 1) * E_CORE)
        x1 = fea_in1[s]
        x2 = fea_in2[s]
        fw = fea_w[s]
        x1s_t = np.zeros((128, E_PAD), np.float32)
        x1s_t[:, :E_CORE] = x1[:, :128].T
        x1v_t = np.zeros((192, E_PAD), np.float32)
        x1v_t[:, :E_CORE] = (
            x1[:, 128:].reshape(E_CORE, 64, 3).transpose(2, 1, 0)
            .reshape(192, E_CORE))
        fw_t = np.zeros((128, E_PAD), np.float32)
        fw_t[:, :E_CORE] = fw.T
        x2p = np.zeros((E_PAD, 4), np.float32)
        x2p[:E_CORE] = x2
        x2_tiled = np.ascontiguousarray(
            x2p.reshape(T_TILES, NT, 4).transpose(0, 2, 1)
            .reshape(T_TILES, 4 * NT))
        shards.append({
            "x1s_t": np.ascontiguousarray(x1s_t),
            "x1v_t": np.ascontiguousarray(x1v_t),
            "fw_t": np.ascontiguousarray(fw_t),
            "x2_tiled": x2_tiled,
        })
    return shards


def run(inputs, trace=False, trace_kwargs=None):
    """Run the kernel; returns (output [E,320] f32, BassKernelResults)."""
    _ensure_repo_on_path()
    from concourse import bass_utils

    nc = _build_nc()
    weights = _fold_weights(inputs)
    shards = _shard_inputs(inputs)
    in_maps = [{**weights, **sh} for sh in shards]

    kwargs = {}
    if trace:
        _install_ntff_hook()
        kwargs.update(trace=True, **(trace_kwargs or {}))
    res = bass_utils.run_bass_kernel_spmd(
        nc, in_maps, core_ids=list(range(N_CORES)), **kwargs)

    out = np.empty((E_FULL, 320), np.float32)
    for c in range(N_CORES):
        o = res.results[c]["out_t"][:, :E_CORE]          # [320, 25000]
        s = slice(c * E_CORE, (c + 1) * E_CORE)
        out[s, :128] = o[:128].T
        out[s, 128:] = (o[128:].reshape(3, 64, E_CORE)
                        .transpose(2, 1, 0).reshape(E_CORE, 192))
    return out, res


def _install_ntff_hook():
    """Shim the missing antenv.axon_hooks so trace=True works under axon."""
    import types
    import antenv
    from concourse import bass_utils
    if "antenv.axon_hooks" in sys.modules:
        return
    mod = types.ModuleType("antenv.axon_hooks")
    _h = [None]
    mod.set_axon_ntff_profile_hook = lambda h: _h.__setitem__(0, h)
    mod.get_axon_ntff_profile_hook = lambda: _h[0]
    sys.modules["antenv.axon_hooks"] = mod
    antenv.axon_hooks = mod
    from trn_agent_boot.trn_boot import _ntff_profile_via_ctypes
    mod.set_axon_ntff_profile_hook(
        _ntff_profile_via_ctypes("/opt/axon/libaxon_pjrt.so"))
    bass_utils.upload_artifacts = lambda tmpdir: tmpdir


def kernel(**inputs) -> np.ndarray:
    out, _ = run(inputs, trace=False)
    return out
